# revision 1
# baseline (speedup 1.0000x reference)
"""MiniAttention Trainium2 Bass kernel.

Problem: B=8, N=1024, C=768, H=12, D=64.
  qkv = x @ w_qkv.T ; q,k,v heads ; S = (q*SCALE) @ k.T per head
  A1 = conv_l-mix over heads ; P = softmax_m(A1) ; A2 = conv_w-mix over heads
  out = (A2 @ v per head) @ w_proj.T + b_proj

Sharding: pure batch-parallel, 1 batch element per NeuronCore (8 cores).

Per-core design (all PE matmuls in bf16, f32 accumulation):
  - Host passes x^T, w_qkv^T, w_proj^T (transposed on host, bf16).
  - QKV as two matmul orientations: q,k in [cout, n] layout, v in [n, cout].
  - Scores per head h, query-block nb: S_h [nb, 1024] in PSUM -> evac bf16.
  - Head-interleave via SBUF->SBUF DMA into groups of 10 queries:
      Sint_g rows = h*10 + r (all 12 heads of 10 queries) -> the 12x12
      conv mixes become single 120x120 sparse matmuls on the PE.
  - conv_l (with SCALE folded) as constant block lhsT; exp on ACT with
    accum_out giving the softmax sums for free; softmax normalization is
    folded into the per-group conv_w lhsT (rows scaled by 1/sum).
  - A2 [rows, m] -> DMA-transpose (HWDGE xbar, bf16) into [m, rows] chunks
    so attention@V contracts m on the PE at K=128.
  - proj consumes the accumulated attn^T [768, 1024]; host re-transposes
    the [768, 1024] per-core output and adds b_proj.
"""

import numpy as np
import ml_dtypes

B, N, C, H = 8, 1024, 768, 12
D = C // H
SCALE = D ** -0.5
G = 10          # queries per mix group
NB = 120        # queries per block (12 groups)
NBLK = 8        # full blocks; last block is ragged: 6 groups of 10 + 1 of 4
BF16 = ml_dtypes.bfloat16

_cached = None


def _block_layout():
    """Returns list of blocks: (n0, nb, chunks) where chunks is a list of
    (row_start, g_start, g_count, g_size) describing the query groups."""
    blocks = []
    for b in range(NBLK):
        blocks.append((b * NB, NB, [(0, 0, 12, G)]))
    # ragged tail: n in [960, 1024) = 6 groups of 10 + 1 group of 4
    blocks.append((960, 64, [(0, 0, 6, G), (60, 6, 1, 4)]))
    return blocks


def _build_program():
    import concourse.tile as tile
    from concourse import bacc, mybir

    f32 = mybir.dt.float32
    bf16 = mybir.dt.bfloat16
    Exp = mybir.ActivationFunctionType.Exp

    nc = bacc.Bacc("TRN2", target_bir_lowering=False, debug=False)

    xt = nc.dram_tensor("xt", [C, N], bf16, kind="ExternalInput").ap()
    wqkvt = nc.dram_tensor("wqkvt", [C, 3 * C], bf16, kind="ExternalInput").ap()
    wprojt = nc.dram_tensor("wprojt", [C, C], bf16, kind="ExternalInput").ap()
    m1w_in = nc.dram_tensor("m1w", [12 * G, 12 * G], bf16, kind="ExternalInput").ap()
    m1w4_in = nc.dram_tensor("m1w4", [12 * 4, 12 * 4], bf16, kind="ExternalInput").ap()
    m2p_in = nc.dram_tensor("m2p", [12 * G, 128], f32, kind="ExternalInput").ap()
    m2p4_in = nc.dram_tensor("m2p4", [12 * 4, 128], f32, kind="ExternalInput").ap()
    out_d = nc.dram_tensor("out", [C, N], f32, kind="ExternalOutput").ap()

    KC = C // 128  # 6 contraction chunks

    # evacuation engine round robin: ~3/8 ACT, 5/8 DVE
    _ec = [0]

    def evac(dst, src):
        i = _ec[0]
        _ec[0] += 1
        if i % 8 < 3:
            nc.scalar.copy(dst, src)
        else:
            nc.vector.tensor_copy(dst, src)

    with tile.TileContext(nc) as tc:
        with tc.tile_pool(name="const", bufs=1) as const, \
             tc.tile_pool(name="big", bufs=1) as big:

            m1wsb = const.tile([120, 120], bf16)
            nc.sync.dma_start(m1wsb, m1w_in)
            m1w4sb = const.tile([48, 48], bf16)
            nc.sync.dma_start(m1w4sb, m1w4_in)
            m2psb = const.tile([120, 128], f32)
            nc.sync.dma_start(m2psb, m2p_in)
            m2p4sb = const.tile([48, 128], f32)
            nc.sync.dma_start(m2p4sb, m2p4_in)

            # persistent activations
            qksb = big.tile([128, 2 * KC, N], bf16)   # ct 0..5 = q, 6..11 = k
            vsb = big.tile([128, 8, C], bf16)         # [m%128, m//128, cout]
            attnT = big.tile([128, KC, N], bf16)      # [cout2%128, cout2//128, n]

            # ---------------- QKV ----------------
            with tc.tile_pool(name="xtp", bufs=1) as xtp, \
                 tc.tile_pool(name="qkvps", bufs=3, space="PSUM") as qkvps, \
                 tc.tile_pool(name="vps", bufs=2, space="PSUM") as vps:
                xtsb = xtp.tile([128, KC, N], bf16)
                nc.sync.dma_start(xtsb, xt.rearrange("(kc p) n -> p kc n", p=128))
                wqsb = xtp.tile([128, KC, 3 * C], bf16)
                nc.sync.dma_start(
                    wqsb, wqkvt.rearrange("(kc p) c -> p kc c", p=128))

                # q, k: out[cout_tile, n]
                for ct in range(12):
                    for nh in range(2):
                        ps = qkvps.tile([128, 512], f32, tag="qkv")
                        for kc in range(KC):
                            nc.tensor.matmul(
                                ps,
                                lhsT=wqsb[:, kc, 128 * ct:128 * ct + 128],
                                rhs=xtsb[:, kc, 512 * nh:512 * nh + 512],
                                start=(kc == 0), stop=(kc == KC - 1),
                            )
                        evac(qksb[:, ct, 512 * nh:512 * nh + 512], ps)

                # v: out[n_tile, cout]
                for nt in range(8):
                    ps = vps.tile([128, 768], f32, tag="vps")
                    for half, (c0, c1) in enumerate([(0, 512), (512, 768)]):
                        for kc in range(KC):
                            nc.tensor.matmul(
                                ps[:, c0:c1],
                                lhsT=xtsb[:, kc, 128 * nt:128 * nt + 128],
                                rhs=wqsb[:, kc, 2 * C + c0:2 * C + c1],
                                start=(kc == 0), stop=(kc == KC - 1),
                            )
                    evac(vsb[:, nt, :], ps)

            # ---------------- attention blocks ----------------
            with tc.tile_pool(name="spl", bufs=3) as spl, \
                 tc.tile_pool(name="sint", bufs=2) as sintp, \
                 tc.tile_pool(name="pint", bufs=3) as pintp, \
                 tc.tile_pool(name="a2p", bufs=3) as a2p, \
                 tc.tile_pool(name="a2tp", bufs=2) as a2tp, \
                 tc.tile_pool(name="smp", bufs=6) as smp, \
                 tc.tile_pool(name="m2wp", bufs=4) as m2wp, \
                 tc.tile_pool(name="psS", bufs=2, space="PSUM") as psS, \
                 tc.tile_pool(name="psA1", bufs=2, space="PSUM") as psA1, \
                 tc.tile_pool(name="psA2", bufs=2, space="PSUM") as psA2, \
                 tc.tile_pool(name="psAV", bufs=2, space="PSUM") as psAV:

                for (n0, nb, chunks) in _block_layout():
                    ngroups = sum(gc for (_, _, gc, _) in chunks)
                    sint = sintp.tile([128, 12, N], bf16, tag="sint")

                    # --- scores per head + interleave DMA ---
                    for h in range(12):
                        base = 64 * (h % 2)
                        sph = spl.tile([128, N], bf16, tag="spl")
                        for mh in range(2):
                            ps = psS.tile([128, 512], f32, tag="psS")
                            nc.tensor.matmul(
                                ps[0:nb, :],
                                lhsT=qksb[base:base + 64, h // 2, n0:n0 + nb],
                                rhs=qksb[base:base + 64, 6 + h // 2,
                                         512 * mh:512 * mh + 512],
                                start=True, stop=True,
                            )
                            evac(sph[0:nb, 512 * mh:512 * mh + 512], ps[0:nb, :])
                        for (rs, g0, gc, gs) in chunks:
                            for gi in range(gc):
                                nc.sync.dma_start(
                                    out=sint[gs * h:gs * h + gs, g0 + gi, :],
                                    in_=sph[rs + gi * gs:rs + (gi + 1) * gs, :],
                                )

                    # --- per-group mix1 -> exp -> mix2 -> transpose ---
                    a2t = a2tp.tile([128, 12, 8, 128], bf16, tag="a2t")
                    for (rs, g0, gc, gs) in chunks:
                        rows = 12 * gs
                        m1 = m1wsb if gs == G else m1w4sb
                        m2 = m2psb if gs == G else m2p4sb
                        for g in range(g0, g0 + gc):
                            pg = pintp.tile([128, N], bf16, tag="pint")
                            sm = smp.tile([128, 4], f32, tag="sm")
                            for mh in range(2):
                                a1 = psA1.tile([128, 512], f32, tag="psA1")
                                nc.tensor.matmul(
                                    a1[0:rows, :],
                                    lhsT=m1,
                                    rhs=sint[0:rows, g, 512 * mh:512 * mh + 512],
                                    start=True, stop=True,
                                )
                                nc.scalar.activation(
                                    pg[0:rows, 512 * mh:512 * mh + 512],
                                    a1[0:rows, :], Exp,
                                    accum_out=sm[0:rows, mh:mh + 1],
                                )
                            nc.vector.tensor_add(
                                sm[0:rows, 2:3], sm[0:rows, 0:1], sm[0:rows, 1:2])
                            nc.vector.reciprocal(
                                sm[0:rows, 3:4], sm[0:rows, 2:3])
                            w2 = m2wp.tile([128, 128], bf16, tag="m2w")
                            nc.vector.tensor_scalar_mul(
                                w2[0:rows, :], m2, sm[0:rows, 3:4])
                            a2sb = a2p.tile([128, N], bf16, tag="a2sb")
                            for mh in range(2):
                                a2 = psA2.tile([128, 512], f32, tag="psA2")
                                nc.tensor.matmul(
                                    a2,
                                    lhsT=w2[0:rows, :],
                                    rhs=pg[0:rows, 512 * mh:512 * mh + 512],
                                    start=True, stop=True,
                                )
                                evac(a2sb[:, 512 * mh:512 * mh + 512], a2)
                            nc.sync.dma_start(
                                out=a2t[:, g, :, :], in_=a2sb,
                                transpose=True,
                            )

                    # --- attention @ V ---
                    for o in range(12):
                        av = psAV.tile([64, 128], f32, tag="psAV")
                        for (rs, g0, gc, gs) in chunks:
                            for c in range(8):
                                nc.tensor.matmul(
                                    av[:, rs:rs + gc * gs],
                                    lhsT=vsb[:, c, 64 * o:64 * o + 64],
                                    rhs=a2t[:, g0:g0 + gc, c,
                                            gs * o:gs * o + gs],
                                    start=(c == 0), stop=(c == 7),
                                )
                        evac(attnT[64 * (o % 2):64 * (o % 2) + 64, o // 2,
                                   n0:n0 + nb],
                             av[:, 0:nb])

            # ---------------- proj ----------------
            with tc.tile_pool(name="projps", bufs=3, space="PSUM") as pjp, \
                 tc.tile_pool(name="outp", bufs=3) as outp:
                wpsb = outp.tile([128, KC, C], bf16)
                nc.sync.dma_start(
                    wpsb, wprojt.rearrange("(kc p) c -> p kc c", p=128))
                od = out_d.rearrange("(ct p) n -> p ct n", p=128)
                for ct in range(KC):
                    for nh in range(2):
                        ps = pjp.tile([128, 512], f32, tag="pj")
                        for kc in range(KC):
                            nc.tensor.matmul(
                                ps,
                                lhsT=wpsb[:, kc, 128 * ct:128 * ct + 128],
                                rhs=attnT[:, kc, 512 * nh:512 * nh + 512],
                                start=(kc == 0), stop=(kc == KC - 1),
                            )
                        ob = outp.tile([128, 512], f32, tag="ob")
                        evac(ob, ps)
                        nc.sync.dma_start(
                            od[:, ct, 512 * nh:512 * nh + 512], ob)

    nc.compile()
    return nc


def _mix_weights(conv_l_w, conv_w_w):
    """Host-built mix lhsT matrices.

    m1w[h*gs+r, o*gs+r2] = (r==r2) * SCALE * conv_l[o, h]   (lhsT for mix1)
    m2p[o*gs+r, o2*gs+r2] = (r==r2) * conv_w[o2, o]         (pattern, f32;
        scaled per-group on device by 1/softmax_sum per row; 128 cols,
        cols >= 12*gs are zero so A2 psum rows [rows:128) are zeros)
    """
    outs = []
    for gs in (G, 4):
        rows = 12 * gs
        m1 = np.zeros((rows, rows), np.float32)
        m2 = np.zeros((rows, 128), np.float32)
        for h in range(12):
            for o in range(12):
                for r in range(gs):
                    m1[h * gs + r, o * gs + r] = SCALE * conv_l_w[o, h]
                    m2[h * gs + r, o * gs + r] = conv_w_w[o, h]
        outs.append((m1.astype(BF16), m2.astype(np.float32)))
    (m1w, m2p), (m1w4, m2p4) = outs
    return m1w, m2p, m1w4, m2p4


def _run(x, w_qkv, w_proj, b_proj, conv_l_w, conv_w_w, **spmd_kwargs):
    global _cached
    from concourse import bass_utils

    x = np.asarray(x, np.float32)
    w_qkv = np.asarray(w_qkv, np.float32)
    w_proj = np.asarray(w_proj, np.float32)
    b_proj = np.asarray(b_proj, np.float32)
    conv_l_w = np.asarray(conv_l_w, np.float32)
    conv_w_w = np.asarray(conv_w_w, np.float32)

    if _cached is None:
        _cached = _build_program()
    nc = _cached

    m1w, m2p, m1w4, m2p4 = _mix_weights(conv_l_w, conv_w_w)
    wqkvt = np.ascontiguousarray(w_qkv.T).astype(BF16)
    wprojt = np.ascontiguousarray(w_proj.T).astype(BF16)

    in_maps = []
    for b in range(B):
        in_maps.append({
            "xt": np.ascontiguousarray(x[b].T).astype(BF16),
            "wqkvt": wqkvt,
            "wprojt": wprojt,
            "m1w": m1w,
            "m1w4": m1w4,
            "m2p": m2p,
            "m2p4": m2p4,
        })

    res = bass_utils.run_bass_kernel_spmd(
        nc, in_maps, core_ids=list(range(B)), **spmd_kwargs)
    out = np.stack([res.results[b]["out"].T for b in range(B)])  # [B, N, C]
    return (out + b_proj[None, None, :]).astype(np.float32), res


def kernel(x, w_qkv, w_proj, b_proj, conv_l_w, conv_w_w):
    out, _ = _run(x, w_qkv, w_proj, b_proj, conv_l_w, conv_w_w)
    return out



# revision 3
# speedup vs baseline: 1.6988x; 1.6988x over previous
"""MiniAttention Trainium2 Bass kernel.

Problem: B=8, N=1024, C=768, H=12, D=64.
  qkv = x @ w_qkv.T ; q,k,v heads ; S = (q*SCALE) @ k.T per head
  A1 = conv_l-mix over heads ; P = softmax_m(A1) ; A2 = conv_w-mix over heads
  out = (A2 @ v per head) @ w_proj.T + b_proj
Sharding: pure batch-parallel, 1 batch element per NeuronCore (8 cores).

Per-core design (PE matmuls in bf16, f32 accumulation):
  - Host passes x^T, w_qkv^T, w_proj^T (transposed on host, bf16).
  - QKV as two matmul orientations: q,k in [cout, n] layout, v in [n, cout].
  - Scores per head h, query-block nb: S_h [nb, 1024] in PSUM -> evac bf16
    into S_all [block_row, (h, m)] (all heads side by side on the free dim).
  - Head-interleave into groups of G=10 queries with row map r = rr*12 + h:
    ONE SBUF->SBUF DMA per group (dest sint[:, g, :], src
    S_all[g*G:(g+1)*G, :, :]) instead of one DMA per (head, group) --
    HWDGE dma_start costs ~600ns of engine occupancy each, so instruction
    count dominates; 13/block instead of 144/block.
  - conv_l (with SCALE folded) as constant rr-block-diagonal lhsT; exp on
    ACT with accum_out giving softmax sums for free; softmax normalization
    folded into the per-group conv_w lhsT (rows scaled by 1/sum).
  - A2 [rows, m] -> DMA-transpose (HWDGE xbar, bf16) into [m, rows] chunks
    so attention@V contracts m on the PE at K=128.
  - proj consumes the accumulated attn^T [768, 1024]; host re-transposes
    the [768, 1024] per-core output and adds b_proj.
"""

import numpy as np
import ml_dtypes

B, N, C, H = 8, 1024, 768, 12
D = C // H
SCALE = D ** -0.5
G = 10          # queries per mix group
NB = 120        # queries per block (12 groups)
NBLK = 8        # full blocks; last block is ragged: 6 groups of 10 + 1 of 4
BF16 = ml_dtypes.bfloat16

_cached = None


def _block_layout():
    """Returns list of blocks: (n0, nb, chunks) where chunks is a list of
    (row_start, g_start, g_count, g_size) describing the query groups."""
    blocks = []
    for b in range(NBLK):
        blocks.append((b * NB, NB, [(0, 0, 12, G)]))
    # ragged tail: n in [960, 1024) = 6 groups of 10 + 1 group of 4
    blocks.append((960, 64, [(0, 0, 6, G), (60, 6, 1, 4)]))
    return blocks


def _build_program():
    import concourse.tile as tile
    from concourse import bacc, mybir

    f32 = mybir.dt.float32
    bf16 = mybir.dt.bfloat16
    Exp = mybir.ActivationFunctionType.Exp

    nc = bacc.Bacc("TRN2", target_bir_lowering=False, debug=False)

    xt = nc.dram_tensor("xt", [C, N], bf16, kind="ExternalInput").ap()
    wqkvt = nc.dram_tensor("wqkvt", [C, 3 * C], bf16, kind="ExternalInput").ap()
    wprojt = nc.dram_tensor("wprojt", [C, C], bf16, kind="ExternalInput").ap()
    m1w_in = nc.dram_tensor("m1w", [12 * G, 12 * G], bf16, kind="ExternalInput").ap()
    m2p_in = nc.dram_tensor("m2p", [12 * G, 128], f32, kind="ExternalInput").ap()
    out_d = nc.dram_tensor("out", [C, N], f32, kind="ExternalOutput").ap()

    KC = C // 128  # 6 contraction chunks

    # evacuation engine round robin: ~3/8 ACT, 5/8 DVE
    _ec = [0]

    def evac(dst, src):
        i = _ec[0]
        _ec[0] += 1
        if i % 8 < 3:
            nc.scalar.copy(dst, src)
        else:
            nc.vector.tensor_copy(dst, src)

    with tile.TileContext(nc) as tc:
        with tc.tile_pool(name="const", bufs=1) as const, \
             tc.tile_pool(name="big", bufs=1) as big:

            m1wsb = const.tile([120, 120], bf16)
            nc.sync.dma_start(m1wsb, m1w_in)
            m2psb = const.tile([120, 128], f32)
            nc.sync.dma_start(m2psb, m2p_in)

            # persistent activations
            qksb = big.tile([128, 2 * KC, N], bf16)   # ct 0..5 = q, 6..11 = k
            vsb = big.tile([128, 8, C], bf16)         # [m%128, m//128, cout]
            attnT = big.tile([128, KC, N], bf16)      # [cout2%128, cout2//128, n]

            # ---------------- QKV ----------------
            with tc.tile_pool(name="xtp", bufs=1) as xtp, \
                 tc.tile_pool(name="qkvps", bufs=3, space="PSUM") as qkvps, \
                 tc.tile_pool(name="vps", bufs=2, space="PSUM") as vps:
                xtsb = xtp.tile([128, KC, N], bf16)
                nc.sync.dma_start(xtsb, xt.rearrange("(kc p) n -> p kc n", p=128))
                wqsb = xtp.tile([128, KC, 3 * C], bf16)
                nc.sync.dma_start(
                    wqsb, wqkvt.rearrange("(kc p) c -> p kc c", p=128))

                # q, k: out[cout_tile, n]
                for ct in range(12):
                    for nh in range(2):
                        ps = qkvps.tile([128, 512], f32, tag="qkv")
                        for kc in range(KC):
                            nc.tensor.matmul(
                                ps,
                                lhsT=wqsb[:, kc, 128 * ct:128 * ct + 128],
                                rhs=xtsb[:, kc, 512 * nh:512 * nh + 512],
                                start=(kc == 0), stop=(kc == KC - 1),
                            )
                        evac(qksb[:, ct, 512 * nh:512 * nh + 512], ps)

                # v: out[n_tile, cout]
                for nt in range(8):
                    ps = vps.tile([128, 768], f32, tag="vps")
                    for half, (c0, c1) in enumerate([(0, 512), (512, 768)]):
                        for kc in range(KC):
                            nc.tensor.matmul(
                                ps[:, c0:c1],
                                lhsT=xtsb[:, kc, 128 * nt:128 * nt + 128],
                                rhs=wqsb[:, kc, 2 * C + c0:2 * C + c1],
                                start=(kc == 0), stop=(kc == KC - 1),
                            )
                    evac(vsb[:, nt, :], ps)

            # ---------------- attention blocks ----------------
            with tc.tile_pool(name="sallp", bufs=2) as sallp, \
                 tc.tile_pool(name="sint", bufs=2) as sintp, \
                 tc.tile_pool(name="pint", bufs=3) as pintp, \
                 tc.tile_pool(name="a2p", bufs=3) as a2p, \
                 tc.tile_pool(name="a2tp", bufs=2) as a2tp, \
                 tc.tile_pool(name="smp", bufs=6) as smp, \
                 tc.tile_pool(name="m2wp", bufs=4) as m2wp, \
                 tc.tile_pool(name="psS", bufs=2, space="PSUM") as psS, \
                 tc.tile_pool(name="psA1", bufs=2, space="PSUM") as psA1, \
                 tc.tile_pool(name="psA2", bufs=2, space="PSUM") as psA2, \
                 tc.tile_pool(name="psAV", bufs=2, space="PSUM") as psAV:

                for (n0, nb, chunks) in _block_layout():
                    sall = sallp.tile([128, 12, N], bf16, tag="sall")

                    # --- scores per head, evac into S_all [row, (h, m)] ---
                    for h in range(12):
                        base = 64 * (h % 2)
                        for mh in range(2):
                            ps = psS.tile([128, 512], f32, tag="psS")
                            nc.tensor.matmul(
                                ps[0:nb, :],
                                lhsT=qksb[base:base + 64, h // 2, n0:n0 + nb],
                                rhs=qksb[base:base + 64, 6 + h // 2,
                                         512 * mh:512 * mh + 512],
                                start=True, stop=True,
                            )
                            evac(sall[0:nb, h, 512 * mh:512 * mh + 512],
                                 ps[0:nb, :])

                    # --- interleave: one DMA per group ---
                    # sint[rr*12+h, g, m] = S_all[g*gs+rr, h, m]
                    sint = sintp.tile([128, 12, N], bf16, tag="sint")
                    for (rs, g0, gc, gs) in chunks:
                        for gi in range(gc):
                            nc.sync.dma_start(
                                out=sint[0:12 * gs, g0 + gi, :],
                                in_=sall[rs + gi * gs:rs + (gi + 1) * gs, :, :],
                            )

                    # --- per-group mix1 -> exp -> mix2 -> transpose ---
                    a2t = a2tp.tile([128, 12, 8, 128], bf16, tag="a2t")
                    for (rs, g0, gc, gs) in chunks:
                        rows = 12 * gs
                        for g in range(g0, g0 + gc):
                            pg = pintp.tile([128, N], bf16, tag="pint")
                            sm = smp.tile([128, 4], f32, tag="sm")
                            for mh in range(2):
                                a1 = psA1.tile([128, 512], f32, tag="psA1")
                                nc.tensor.matmul(
                                    a1[0:rows, :],
                                    lhsT=m1wsb[0:rows, 0:rows],
                                    rhs=sint[0:rows, g, 512 * mh:512 * mh + 512],
                                    start=True, stop=True,
                                )
                                nc.scalar.activation(
                                    pg[0:rows, 512 * mh:512 * mh + 512],
                                    a1[0:rows, :], Exp,
                                    accum_out=sm[0:rows, mh:mh + 1],
                                )
                            nc.vector.tensor_add(
                                sm[0:rows, 2:3], sm[0:rows, 0:1], sm[0:rows, 1:2])
                            nc.vector.reciprocal(
                                sm[0:rows, 3:4], sm[0:rows, 2:3])
                            w2 = m2wp.tile([128, 128], bf16, tag="m2w")
                            nc.vector.tensor_scalar_mul(
                                w2[0:rows, :], m2psb[0:rows, :], sm[0:rows, 3:4])
                            a2sb = a2p.tile([128, N], bf16, tag="a2sb")
                            for mh in range(2):
                                a2 = psA2.tile([128, 512], f32, tag="psA2")
                                nc.tensor.matmul(
                                    a2,
                                    lhsT=w2[0:rows, :],
                                    rhs=pg[0:rows, 512 * mh:512 * mh + 512],
                                    start=True, stop=True,
                                )
                                evac(a2sb[:, 512 * mh:512 * mh + 512], a2)
                            nc.sync.dma_start(
                                out=a2t[:, g, :, :], in_=a2sb,
                                transpose=True,
                            )

                    # --- attention @ V ---
                    # A2 row (rr, o) = rr*12 + o; head-o rows = o::12
                    for o in range(12):
                        av = psAV.tile([64, 128], f32, tag="psAV")
                        for (rs, g0, gc, gs) in chunks:
                            for c in range(8):
                                nc.tensor.matmul(
                                    av[:, rs:rs + gc * gs],
                                    lhsT=vsb[:, c, 64 * o:64 * o + 64],
                                    rhs=a2t[:, g0:g0 + gc, c,
                                            o:o + 12 * (gs - 1) + 1:12],
                                    start=(c == 0), stop=(c == 7),
                                )
                        evac(attnT[64 * (o % 2):64 * (o % 2) + 64, o // 2,
                                   n0:n0 + nb],
                             av[:, 0:nb])

            # ---------------- proj ----------------
            with tc.tile_pool(name="projps", bufs=3, space="PSUM") as pjp, \
                 tc.tile_pool(name="outp", bufs=3) as outp:
                wpsb = outp.tile([128, KC, C], bf16)
                nc.sync.dma_start(
                    wpsb, wprojt.rearrange("(kc p) c -> p kc c", p=128))
                od = out_d.rearrange("(ct p) n -> p ct n", p=128)
                for ct in range(KC):
                    for nh in range(2):
                        ps = pjp.tile([128, 512], f32, tag="pj")
                        for kc in range(KC):
                            nc.tensor.matmul(
                                ps,
                                lhsT=wpsb[:, kc, 128 * ct:128 * ct + 128],
                                rhs=attnT[:, kc, 512 * nh:512 * nh + 512],
                                start=(kc == 0), stop=(kc == KC - 1),
                            )
                        ob = outp.tile([128, 512], f32, tag="ob")
                        evac(ob, ps)
                        nc.sync.dma_start(
                            od[:, ct, 512 * nh:512 * nh + 512], ob)

    nc.compile()
    return nc


def _mix_weights(conv_l_w, conv_w_w):
    """Host-built mix lhsT matrices, row map r = rr*12 + h.

    m1w[rr*12+h, rr*12+o] = SCALE * conv_l[o, h]   (lhsT for mix1)
    m2p[rr*12+h, rr*12+o] = conv_w[o, h]           (pattern, f32; scaled
        per-group on device by 1/softmax_sum per row; 128 cols, cols >= 120
        are zero so A2 psum rows [rows:128) are zeros)
    The gs=4 ragged group uses the leading [48, 48] / [48, :] slices.
    """
    m1 = np.zeros((120, 120), np.float32)
    m2 = np.zeros((120, 128), np.float32)
    for rr in range(G):
        for h in range(12):
            for o in range(12):
                m1[rr * 12 + h, rr * 12 + o] = SCALE * conv_l_w[o, h]
                m2[rr * 12 + h, rr * 12 + o] = conv_w_w[o, h]
    return m1.astype(BF16), m2.astype(np.float32)


def _run(x, w_qkv, w_proj, b_proj, conv_l_w, conv_w_w, **spmd_kwargs):
    global _cached
    from concourse import bass_utils

    x = np.asarray(x, np.float32)
    w_qkv = np.asarray(w_qkv, np.float32)
    w_proj = np.asarray(w_proj, np.float32)
    b_proj = np.asarray(b_proj, np.float32)
    conv_l_w = np.asarray(conv_l_w, np.float32)
    conv_w_w = np.asarray(conv_w_w, np.float32)

    if _cached is None:
        _cached = _build_program()
    nc = _cached

    m1w, m2p = _mix_weights(conv_l_w, conv_w_w)
    wqkvt = np.ascontiguousarray(w_qkv.T).astype(BF16)
    wprojt = np.ascontiguousarray(w_proj.T).astype(BF16)

    in_maps = []
    for b in range(B):
        in_maps.append({
            "xt": np.ascontiguousarray(x[b].T).astype(BF16),
            "wqkvt": wqkvt,
            "wprojt": wprojt,
            "m1w": m1w,
            "m2p": m2p,
        })

    res = bass_utils.run_bass_kernel_spmd(
        nc, in_maps, core_ids=list(range(B)), **spmd_kwargs)
    out = np.stack([res.results[b]["out"].T for b in range(B)])  # [B, N, C]
    return (out + b_proj[None, None, :]).astype(np.float32), res


def kernel(x, w_qkv, w_proj, b_proj, conv_l_w, conv_w_w):
    out, _ = _run(x, w_qkv, w_proj, b_proj, conv_l_w, conv_w_w)
    return out


# revision 5
# speedup vs baseline: 1.9552x; 1.1509x over previous
"""MiniAttention Trainium2 Bass kernel.

Problem: B=8, N=1024, C=768, H=12, D=64.
  qkv = x @ w_qkv.T ; q,k,v heads ; S = (q*SCALE) @ k.T per head
  A1 = conv_l-mix over heads ; P = softmax_m(A1) ; A2 = conv_w-mix over heads
  out = (A2 @ v per head) @ w_proj.T + b_proj
Sharding: pure batch-parallel, 1 batch element per NeuronCore (8 cores).

Per-core design (PE matmuls in bf16, f32 accumulation):
  - Host passes x^T, w_qkv^T, w_proj^T (transposed on host, bf16).
  - QKV as two matmul orientations: q,k in [cout, n] layout, v in [n, cout].
  - Scores per head h, query-block nb: S_h [nb, 1024] in PSUM -> evac bf16
    into S_all [block_row, (h, m)] (all heads side by side on the free dim).
  - Head-interleave into groups of G=10 queries with row map r = rr*12 + h:
    ONE SBUF->SBUF DMA per group (dest sint[:, g, :], src
    S_all[g*G:(g+1)*G, :, :]) instead of one DMA per (head, group) --
    HWDGE dma_start costs ~600ns of engine occupancy each, so instruction
    count dominates; 13/block instead of 144/block.
  - conv_l (with SCALE folded) as constant rr-block-diagonal lhsT; exp on
    ACT with accum_out giving softmax sums for free; softmax normalization
    folded into the per-group conv_w lhsT (rows scaled by 1/sum).
  - A2 [rows, m] -> DMA-transpose (HWDGE xbar, bf16) into [m, rows] chunks
    so attention@V contracts m on the PE at K=128.
  - proj consumes the accumulated attn^T [768, 1024]; host re-transposes
    the [768, 1024] per-core output and adds b_proj.
"""

import numpy as np
import ml_dtypes

B, N, C, H = 8, 1024, 768, 12
D = C // H
SCALE = D ** -0.5
G = 10          # queries per mix group
NB = 120        # queries per block (12 groups)
NBLK = 8        # full blocks; last block is ragged: 6 groups of 10 + 1 of 4
BF16 = ml_dtypes.bfloat16

_cached = None


def _block_layout():
    """Returns list of blocks: (n0, nb, chunks) where chunks is a list of
    (row_start, g_start, g_count, g_size) describing the query groups."""
    blocks = []
    for b in range(NBLK):
        blocks.append((b * NB, NB, [(0, 0, 12, G)]))
    # ragged tail: n in [960, 1024) = 6 groups of 10 + 1 group of 4
    blocks.append((960, 64, [(0, 0, 6, G), (60, 6, 1, 4)]))
    return blocks


def _build_program():
    import concourse.tile as tile
    from concourse import bacc, mybir

    f32 = mybir.dt.float32
    bf16 = mybir.dt.bfloat16
    Exp = mybir.ActivationFunctionType.Exp

    nc = bacc.Bacc("TRN2", target_bir_lowering=False, debug=False)

    xt = nc.dram_tensor("xt", [C, N], bf16, kind="ExternalInput").ap()
    wqkvt = nc.dram_tensor("wqkvt", [C, 3 * C], bf16, kind="ExternalInput").ap()
    wprojt = nc.dram_tensor("wprojt", [C, C], bf16, kind="ExternalInput").ap()
    m1w_in = nc.dram_tensor("m1w", [12 * G, 12 * G], bf16, kind="ExternalInput").ap()
    m2p_in = nc.dram_tensor("m2p", [12 * G, 128], f32, kind="ExternalInput").ap()
    out_d = nc.dram_tensor("out", [C, N], f32, kind="ExternalOutput").ap()

    KC = C // 128  # 6 contraction chunks

    # evacuation engine round robin: ~3/8 ACT, 5/8 DVE
    _ec = [0]

    def evac(dst, src):
        i = _ec[0]
        _ec[0] += 1
        if i % 8 < 3:
            nc.scalar.copy(dst, src)
        else:
            nc.vector.tensor_copy(dst, src)

    with tile.TileContext(nc) as tc:
        with tc.tile_pool(name="const", bufs=1) as const, \
             tc.tile_pool(name="big", bufs=1) as big:

            m1wsb = const.tile([120, 120], bf16)
            nc.sync.dma_start(m1wsb, m1w_in)
            m2psb = const.tile([120, 128], f32)
            nc.sync.dma_start(m2psb, m2p_in)

            # persistent activations
            qksb = big.tile([128, 2 * KC, N], bf16)   # ct 0..5 = q, 6..11 = k
            vsb = big.tile([128, 8, C], bf16)         # [m%128, m//128, cout]
            attnT = big.tile([128, KC, N], bf16)      # [cout2%128, cout2//128, n]

            # ---------------- QKV ----------------
            with tc.tile_pool(name="xtp", bufs=1) as xtp, \
                 tc.tile_pool(name="qkvps", bufs=3, space="PSUM") as qkvps, \
                 tc.tile_pool(name="vps", bufs=2, space="PSUM") as vps:
                xtsb = xtp.tile([128, KC, N], bf16)
                nc.sync.dma_start(xtsb, xt.rearrange("(kc p) n -> p kc n", p=128))
                wqsb = xtp.tile([128, KC, 3 * C], bf16)
                nc.sync.dma_start(
                    wqsb, wqkvt.rearrange("(kc p) c -> p kc c", p=128))

                # q, k: out[cout_tile, n]
                for ct in range(12):
                    for nh in range(2):
                        ps = qkvps.tile([128, 512], f32, tag="qkv")
                        for kc in range(KC):
                            nc.tensor.matmul(
                                ps,
                                lhsT=wqsb[:, kc, 128 * ct:128 * ct + 128],
                                rhs=xtsb[:, kc, 512 * nh:512 * nh + 512],
                                start=(kc == 0), stop=(kc == KC - 1),
                            )
                        evac(qksb[:, ct, 512 * nh:512 * nh + 512], ps)

                # v: out[n_tile, cout]
                for nt in range(8):
                    ps = vps.tile([128, 768], f32, tag="vps")
                    for half, (c0, c1) in enumerate([(0, 512), (512, 768)]):
                        for kc in range(KC):
                            nc.tensor.matmul(
                                ps[:, c0:c1],
                                lhsT=xtsb[:, kc, 128 * nt:128 * nt + 128],
                                rhs=wqsb[:, kc, 2 * C + c0:2 * C + c1],
                                start=(kc == 0), stop=(kc == KC - 1),
                            )
                    evac(vsb[:, nt, :], ps)

            # ---------------- attention blocks ----------------
            with tc.tile_pool(name="sallp", bufs=2) as sallp, \
                 tc.tile_pool(name="sint", bufs=2) as sintp, \
                 tc.tile_pool(name="pint", bufs=3) as pintp, \
                 tc.tile_pool(name="a2p", bufs=3) as a2p, \
                 tc.tile_pool(name="a2tp", bufs=2) as a2tp, \
                 tc.tile_pool(name="smp", bufs=6) as smp, \
                 tc.tile_pool(name="m2wp", bufs=4) as m2wp, \
                 tc.tile_pool(name="psS", bufs=2, space="PSUM") as psS, \
                 tc.tile_pool(name="psA1", bufs=2, space="PSUM") as psA1, \
                 tc.tile_pool(name="psA2", bufs=2, space="PSUM") as psA2, \
                 tc.tile_pool(name="psAV", bufs=2, space="PSUM") as psAV:

                for (n0, nb, chunks) in _block_layout():
                    sall = sallp.tile([128, 12, N], bf16, tag="sall")

                    # --- scores per head, evac into S_all [row, (h, m)] ---
                    for h in range(12):
                        base = 64 * (h % 2)
                        for mh in range(2):
                            ps = psS.tile([128, 512], f32, tag="psS")
                            nc.tensor.matmul(
                                ps[0:nb, :],
                                lhsT=qksb[base:base + 64, h // 2, n0:n0 + nb],
                                rhs=qksb[base:base + 64, 6 + h // 2,
                                         512 * mh:512 * mh + 512],
                                start=True, stop=True,
                            )
                            evac(sall[0:nb, h, 512 * mh:512 * mh + 512],
                                 ps[0:nb, :])

                    # --- interleave: one DMA per group ---
                    # sint[rr*12+h, g, m] = S_all[g*gs+rr, h, m]
                    sint = sintp.tile([128, 12, N], bf16, tag="sint")
                    for (rs, g0, gc, gs) in chunks:
                        for gi in range(gc):
                            # SWDGE (gpsimd): keeps the HWDGE/Sync queue free
                            # for the xbar transposes; Pool engine is idle.
                            nc.gpsimd.dma_start(
                                out=sint[0:12 * gs, g0 + gi, :],
                                in_=sall[rs + gi * gs:rs + (gi + 1) * gs, :, :],
                            )

                    # --- per-group mix1 -> exp -> mix2 -> transpose ---
                    a2t = a2tp.tile([128, 12, 8, 128], bf16, tag="a2t")
                    for (rs, g0, gc, gs) in chunks:
                        rows = 12 * gs
                        for g in range(g0, g0 + gc):
                            pg = pintp.tile([128, N], bf16, tag="pint")
                            sm = smp.tile([128, 4], f32, tag="sm")
                            for mh in range(2):
                                a1 = psA1.tile([128, 512], f32, tag="psA1")
                                nc.tensor.matmul(
                                    a1[0:rows, :],
                                    lhsT=m1wsb[0:rows, 0:rows],
                                    rhs=sint[0:rows, g, 512 * mh:512 * mh + 512],
                                    start=True, stop=True,
                                )
                                nc.scalar.activation(
                                    pg[0:rows, 512 * mh:512 * mh + 512],
                                    a1[0:rows, :], Exp,
                                    accum_out=sm[0:rows, mh:mh + 1],
                                )
                            nc.vector.tensor_add(
                                sm[0:rows, 2:3], sm[0:rows, 0:1], sm[0:rows, 1:2])
                            nc.vector.reciprocal(
                                sm[0:rows, 3:4], sm[0:rows, 2:3])
                            w2 = m2wp.tile([128, 128], bf16, tag="m2w")
                            nc.vector.tensor_scalar_mul(
                                w2[0:rows, :], m2psb[0:rows, :], sm[0:rows, 3:4])
                            a2sb = a2p.tile([128, N], bf16, tag="a2sb")
                            for mh in range(2):
                                a2 = psA2.tile([128, 512], f32, tag="psA2")
                                nc.tensor.matmul(
                                    a2,
                                    lhsT=w2[0:rows, :],
                                    rhs=pg[0:rows, 512 * mh:512 * mh + 512],
                                    start=True, stop=True,
                                )
                                evac(a2sb[:, 512 * mh:512 * mh + 512], a2)
                            nc.sync.dma_start(
                                out=a2t[:, g, :, :], in_=a2sb,
                                transpose=True,
                            )

                    # --- attention @ V ---
                    # A2 row (rr, o) = rr*12 + o; head-o rows = o::12.
                    # Column-packed head pairs: head 2j -> av2[0:64] (col
                    # tile 0), head 2j+1 -> av2[64:128] (col tile 64); the
                    # two K=128 chains run concurrently on the PE array.
                    for j in range(6):
                        av2 = psAV.tile([128, 128], f32, tag="psAV")
                        for half in range(2):
                            o = 2 * j + half
                            for (rs, g0, gc, gs) in chunks:
                                for c in range(8):
                                    nc.tensor.matmul(
                                        av2[64 * half:64 * half + 64,
                                            rs:rs + gc * gs],
                                        lhsT=vsb[:, c, 64 * o:64 * o + 64],
                                        rhs=a2t[:, g0:g0 + gc, c,
                                                o:o + 12 * (gs - 1) + 1:12],
                                        start=(c == 0), stop=(c == 7),
                                    )
                        evac(attnT[:, j, n0:n0 + nb], av2[:, 0:nb])

            # ---------------- proj ----------------
            with tc.tile_pool(name="projps", bufs=3, space="PSUM") as pjp, \
                 tc.tile_pool(name="outp", bufs=3) as outp:
                wpsb = outp.tile([128, KC, C], bf16)
                nc.sync.dma_start(
                    wpsb, wprojt.rearrange("(kc p) c -> p kc c", p=128))
                od = out_d.rearrange("(ct p) n -> p ct n", p=128)
                for ct in range(KC):
                    for nh in range(2):
                        ps = pjp.tile([128, 512], f32, tag="pj")
                        for kc in range(KC):
                            nc.tensor.matmul(
                                ps,
                                lhsT=wpsb[:, kc, 128 * ct:128 * ct + 128],
                                rhs=attnT[:, kc, 512 * nh:512 * nh + 512],
                                start=(kc == 0), stop=(kc == KC - 1),
                            )
                        ob = outp.tile([128, 512], f32, tag="ob")
                        evac(ob, ps)
                        nc.sync.dma_start(
                            od[:, ct, 512 * nh:512 * nh + 512], ob)

    nc.compile()
    return nc


def _mix_weights(conv_l_w, conv_w_w):
    """Host-built mix lhsT matrices, row map r = rr*12 + h.

    m1w[rr*12+h, rr*12+o] = SCALE * conv_l[o, h]   (lhsT for mix1)
    m2p[rr*12+h, rr*12+o] = conv_w[o, h]           (pattern, f32; scaled
        per-group on device by 1/softmax_sum per row; 128 cols, cols >= 120
        are zero so A2 psum rows [rows:128) are zeros)
    The gs=4 ragged group uses the leading [48, 48] / [48, :] slices.
    """
    m1 = np.zeros((120, 120), np.float32)
    m2 = np.zeros((120, 128), np.float32)
    for rr in range(G):
        for h in range(12):
            for o in range(12):
                m1[rr * 12 + h, rr * 12 + o] = SCALE * conv_l_w[o, h]
                m2[rr * 12 + h, rr * 12 + o] = conv_w_w[o, h]
    return m1.astype(BF16), m2.astype(np.float32)


def _run(x, w_qkv, w_proj, b_proj, conv_l_w, conv_w_w, **spmd_kwargs):
    global _cached
    from concourse import bass_utils

    x = np.asarray(x, np.float32)
    w_qkv = np.asarray(w_qkv, np.float32)
    w_proj = np.asarray(w_proj, np.float32)
    b_proj = np.asarray(b_proj, np.float32)
    conv_l_w = np.asarray(conv_l_w, np.float32)
    conv_w_w = np.asarray(conv_w_w, np.float32)

    if _cached is None:
        _cached = _build_program()
    nc = _cached

    m1w, m2p = _mix_weights(conv_l_w, conv_w_w)
    wqkvt = np.ascontiguousarray(w_qkv.T).astype(BF16)
    wprojt = np.ascontiguousarray(w_proj.T).astype(BF16)

    in_maps = []
    for b in range(B):
        in_maps.append({
            "xt": np.ascontiguousarray(x[b].T).astype(BF16),
            "wqkvt": wqkvt,
            "wprojt": wprojt,
            "m1w": m1w,
            "m2p": m2p,
        })

    res = bass_utils.run_bass_kernel_spmd(
        nc, in_maps, core_ids=list(range(B)), **spmd_kwargs)
    out = np.stack([res.results[b]["out"].T for b in range(B)])  # [B, N, C]
    return (out + b_proj[None, None, :]).astype(np.float32), res


def kernel(x, w_qkv, w_proj, b_proj, conv_l_w, conv_w_w):
    out, _ = _run(x, w_qkv, w_proj, b_proj, conv_l_w, conv_w_w)
    return out
